# revision 4
# baseline (speedup 1.0000x reference)
"""v3: query-compacted + key-compacted sparse causal attention.

Sharding: 8 cores = 4 batches x 2 key-parity shards.  Host compacts BOTH
the live queries (q_mask kills ~50% of output rows -> never computed) and
the live keys (v_mask, split even/odd between the 2 cores of a batch).

Device per core:
  scores  s[kb] = kt[:,kb]T @ qt[:,sb]    (f32r, out [128k, 256q])
  exp     p = exp(s)                       (Act, bf16 out)
  causal  p *= (qo_rel >= k_orig - base)   (stt on DVE/Pool, only the
                                            column range that can violate)
  PV      o[128q, 65] += p_subT @ vp[kb]   (bf16, out free = 65)
Output per superblock: [128q, 2*65] = (v-dot | l) for 2 query subblocks;
host sums the 2 parity cores, divides by l, scatters to live rows.
"""

import math
import sys

import numpy as np

try:
    import concourse  # noqa: F401
except ImportError:  # pragma: no cover
    sys.path.insert(0, "/opt/trn_rl_repo")

import ml_dtypes

B, T, D = 4, 4096, 64
NCORES = 8
QSB = 256
KB = 128
VW = 65
GRP = 6
NEG_BIG = 1e9

_compiled = {}
_last_plan = None


def _build_nc(key):
    import concourse.bass as bass
    import concourse.mybir as mybir
    import concourse.tile as tile
    from concourse import bacc

    nsb, blocks, slots, copy_eng, qo16 = key
    f32 = mybir.dt.float32
    f32r = mybir.dt.float32r
    bf16 = mybir.dt.bfloat16
    qodt = mybir.dt.float16 if qo16 else f32
    nb_tot = blocks[-1]
    Tqpad = nsb * QSB
    ns_tot = max(1, sum(len(s) for s in slots))

    nc = bacc.Bacc(None, target_bir_lowering=False, debug=False)
    qt_d = nc.declare_dram_parameter("qt", [D, Tqpad], f32r, isOutput=False)
    kt_d = nc.declare_dram_parameter("kt", [D, nb_tot * KB], f32r,
                                     isOutput=False)
    vp_d = nc.declare_dram_parameter("vp", [KB, nb_tot * VW], bf16,
                                     isOutput=False)
    qo_d = nc.declare_dram_parameter("qo", [KB, Tqpad], qodt, isOutput=False)
    th_d = nc.declare_dram_parameter("th", [KB, ns_tot], f32, isOutput=False)
    o_d = nc.declare_dram_parameter("o", [KB, nsb * 2 * VW], f32,
                                    isOutput=True)

    # th column index per (superblock, key-block)
    thcol = {}
    c = 0
    for i in range(nsb):
        for (kb, _chi, _eng) in slots[i]:
            thcol[(i, kb)] = c
            c += 1

    if nsb <= 2:
        order = list(range(nsb))
    else:
        order = [0, 1] + list(range(nsb - 1, 1, -1))

    with tile.TileContext(nc) as tc:
        with (
            tc.tile_pool(name="const", bufs=1) as cpool,
            tc.tile_pool(name="pt", bufs=6) as ppool,
            tc.tile_pool(name="ob", bufs=3) as obpool,
            tc.tile_pool(name="ps", bufs=2, space=bass.MemorySpace.PSUM) as spool,
            tc.tile_pool(name="po", bufs=2, space=bass.MemorySpace.PSUM) as opool,
        ):
            qt = cpool.tile([D, Tqpad], f32r)
            kt = cpool.tile([D, nb_tot * KB], f32r)
            vp = cpool.tile([KB, nb_tot * VW], bf16)
            qo = cpool.tile([KB, Tqpad], qodt)
            th = cpool.tile([KB, ns_tot], f32)

            # order DMAs so superblocks 0/1 can start early
            nA = min(2, nsb)
            ktA = blocks[nA - 1] * KB
            nc.sync.dma_start(kt[:, 0:ktA], kt_d[:, 0:ktA])
            nc.sync.dma_start(qt[:, 0:nA * QSB], qt_d[:, 0:nA * QSB])
            nc.sync.dma_start(th[:], th_d[:])
            nc.sync.dma_start(qo[:, 0:nA * QSB], qo_d[:, 0:nA * QSB])
            nc.sync.dma_start(vp[:], vp_d[:])
            if ktA < nb_tot * KB:
                nc.sync.dma_start(kt[:, ktA:], kt_d[:, ktA:])
            if nsb > nA:
                nc.sync.dma_start(qt[:, nA * QSB:], qt_d[:, nA * QSB:])
                nc.sync.dma_start(qo[:, nA * QSB:], qo_d[:, nA * QSB:])

            # (superblock, first kb, group size, total blocks)
            items = []
            for i in order:
                nb = blocks[i]
                ng = -(-nb // GRP)
                bsz, rem = divmod(nb, ng)
                sizes = [bsz + 1] * rem + [bsz] * (ng - rem)
                kb0 = 0
                for sz in sizes:
                    items.append((i, kb0, sz, nb))
                    kb0 += sz

            o_ps = {}
            pending = None

            def emit_pv(i, kb0, sz, nb, p):
                for j in range(sz):
                    kb = kb0 + j
                    for sub in range(2):
                        nc.tensor.matmul(
                            o_ps[i][:, sub * VW:(sub + 1) * VW],
                            p[:, j * QSB + sub * KB:j * QSB + (sub + 1) * KB],
                            vp[:, kb * VW:(kb + 1) * VW],
                            # start zeroes the whole 2KB zero-region (bank):
                            # one accumulation group per o tile
                            start=(kb == 0 and sub == 0),
                            stop=(kb == nb - 1 and sub == 1),
                        )
                if kb0 + sz == nb:
                    ob = obpool.tile([KB, 2 * VW], f32, name=f"ob{i}",
                                     tag="ob")
                    nc.vector.tensor_copy(ob[:], o_ps[i][:])
                    nc.sync.dma_start(o_d[:, i * 2 * VW:(i + 1) * 2 * VW],
                                      ob[:])
                    del o_ps[i]

            for (i, kb0, sz, nb) in items:
                if kb0 == 0:
                    o_ps[i] = opool.tile([KB, 2 * VW], f32,
                                         name=f"oacc{i}", tag="oacc")
                w = sz * QSB
                s = spool.tile([KB, GRP * QSB], f32)
                for j in range(sz):
                    kb = kb0 + j
                    nc.tensor.matmul(
                        s[:, j * QSB:(j + 1) * QSB],
                        kt[:, kb * KB:(kb + 1) * KB],
                        qt[:, i * QSB:(i + 1) * QSB],
                        start=True, stop=True,
                    )
                if pending is not None:
                    emit_pv(*pending)
                p = ppool.tile([KB, GRP * QSB], bf16)
                nc.scalar.activation(
                    p[:, 0:w], s[:, 0:w],
                    mybir.ActivationFunctionType.Exp,
                )
                for (kb, chi, eng) in slots[i]:
                    if kb0 <= kb < kb0 + sz:
                        j = kb - kb0
                        engine = nc.vector if eng == 0 else nc.gpsimd
                        col = thcol[(i, kb)]
                        engine.scalar_tensor_tensor(
                            p[:, j * QSB:j * QSB + chi],
                            qo[:, i * QSB:i * QSB + chi],
                            th[:, col:col + 1],
                            p[:, j * QSB:j * QSB + chi],
                            op0=mybir.AluOpType.is_ge,
                            op1=mybir.AluOpType.mult,
                        )
                pending = (i, kb0, sz, nb, p)
            emit_pv(*pending)

    nc.compile()
    return nc


def _get_nc(key):
    if key not in _compiled:
        _compiled[key] = _build_nc(key)
    return _compiled[key]


def _host_inputs(query, value, keys, q_mask, v_mask, scale):
    global _last_plan
    scale = np.float32(scale)
    q = np.asarray(query, np.float32)
    v = np.asarray(value, np.float32)
    k = np.asarray(keys, np.float32)
    qm = np.asarray(q_mask).astype(bool)
    vm = np.asarray(v_mask).astype(bool)

    Lqs = [np.flatnonzero(qm[b]) for b in range(B)]
    nlqs = [len(x) for x in Lqs]
    Tqpad = max(QSB, -(-max(nlqs) // QSB) * QSB)
    nsb = Tqpad // QSB
    lives = []
    for c in range(NCORES):
        b, par = c // 2, c % 2
        lives.append(np.flatnonzero(vm[b])[par::2])

    # per-core packed keys: only those visible to some live query
    packed = []
    for c in range(NCORES):
        b = c // 2
        live = lives[c]
        ncnt = int(np.searchsorted(live, Lqs[b][-1] + 1)) if nlqs[b] else 0
        packed.append(live[:ncnt])

    # per-core, per-superblock base/top q_orig and key count
    base = np.full((NCORES, nsb), -1, np.int64)      # -1: no real cols
    cnt = np.zeros((NCORES, nsb), np.int64)
    for c in range(NCORES):
        b = c // 2
        Lq, nlq = Lqs[b], nlqs[b]
        for i in range(nsb):
            j0 = i * QSB
            if j0 < nlq:
                base[c, i] = Lq[j0]
                top = Lq[min((i + 1) * QSB, nlq) - 1]
                cnt[c, i] = np.searchsorted(packed[c], top + 1)
            else:
                cnt[c, i] = cnt[c, i - 1] if i else 0

    blocks = []
    for i in range(nsb):
        nb = max(1, int(max(-(-cnt[c, i] // KB) for c in range(NCORES))))
        if blocks:
            nb = max(nb, blocks[-1])
        blocks.append(nb)
    nb_tot = blocks[-1]
    npad = nb_tot * KB

    # causal-boundary slots: walk blocks from the top; a block is clean when
    # for every core all its real keys are <= that core's superblock base.
    slots = []
    for i in range(nsb):
        sl = []
        for kb in range(blocks[i] - 1, -1, -1):
            allvis = True
            chi = 0
            for c in range(NCORES):
                b = c // 2
                seg = packed[c][kb * KB:(kb + 1) * KB]
                if seg.size == 0 or base[c, i] < 0:
                    continue
                kmax = int(seg[-1])
                if kmax > base[c, i]:
                    allvis = False
                Lq_sb = Lqs[b][i * QSB:min((i + 1) * QSB, nlqs[b])]
                chi = max(chi, int(np.searchsorted(Lq_sb, kmax)))
            if chi > 0:
                sl.append((kb, chi))
            if allvis:
                break
        slots.append(tuple(reversed(sl)))

    # all mask ops on DVE (neuronxcc rejects TensorScalarPtr on Pool/GPSIMD)
    slots = tuple(tuple((kb, chi, 0) for kb, chi in sl) for sl in slots)
    copy_eng = tuple(0 for _ in range(nsb))

    # qo_rel: q_orig - superblock base, fp16 when exactly representable
    qo_rows = np.zeros((NCORES, Tqpad), np.float32)
    for c in range(NCORES):
        b = c // 2
        Lq, nlq = Lqs[b], nlqs[b]
        for i in range(nsb):
            j0, j1 = i * QSB, min((i + 1) * QSB, nlq)
            if j0 < nlq:
                qo_rows[c, j0:j1] = Lq[j0:j1] - base[c, i]
    qo16 = bool((qo_rows.astype(np.float16).astype(np.float32)
                 == qo_rows).all())
    key = (nsb, tuple(blocks), slots, copy_eng, qo16)

    ns_tot = max(1, sum(len(s) for s in slots))
    in_maps = []
    for c in range(NCORES):
        b = c // 2
        pk = packed[c]
        ncnt = len(pk)
        k_orig = np.full(npad, T, np.float32)
        k_orig[:ncnt] = pk
        kc = np.zeros((npad, D), np.float32)
        kc[:ncnt] = k[b][pk]
        vc = np.zeros((npad, VW), np.float32)
        vc[:ncnt, :D] = v[b][pk]
        vc[:ncnt, D] = 1.0
        qt = np.zeros((D, Tqpad), np.float32)
        if nlqs[b]:
            qt[:, :nlqs[b]] = (q[b][Lqs[b]] * scale).T
        kt = np.ascontiguousarray(kc.T)
        vp = np.ascontiguousarray(
            vc.reshape(nb_tot, KB, VW).transpose(1, 0, 2).reshape(KB, -1)
        ).astype(ml_dtypes.bfloat16)
        qo = np.broadcast_to(qo_rows[c], (KB, Tqpad))
        qo = np.ascontiguousarray(
            qo.astype(np.float16) if qo16 else qo)
        th = np.zeros((KB, ns_tot), np.float32)
        col = 0
        for i in range(nsb):
            for (kb, _chi, _eng) in slots[i]:
                bs = base[c, i] if base[c, i] >= 0 else T
                th[:, col] = k_orig[kb * KB:(kb + 1) * KB] - np.float32(bs)
                col += 1
        in_maps.append({"qt": qt, "kt": kt, "vp": vp, "qo": qo, "th": th})

    _last_plan = {"Lqs": Lqs, "lives": lives, "packed": packed,
                  "blocks": blocks, "slots": slots, "nsb": nsb,
                  "Tqpad": Tqpad, "base": base, "cnt": cnt}
    return in_maps, key


def _host_gather(results, query, value, keys, q_mask, v_mask, scale):
    q = np.asarray(query, np.float32)
    v = np.asarray(value, np.float32)
    k = np.asarray(keys, np.float32)
    qm = np.asarray(q_mask).astype(bool)
    vm = np.asarray(v_mask).astype(bool)
    scale = np.float32(scale)
    plan = _last_plan
    nsb = plan["nsb"]

    out = np.zeros((B, T, D), np.float32)
    for b in range(B):
        Lq = plan["Lqs"][b]
        nlq = len(Lq)
        if nlq == 0:
            continue
        osum = results[2 * b]["o"].astype(np.float32) \
            + results[2 * b + 1]["o"].astype(np.float32)
        arr = osum.reshape(KB, nsb, 2, VW).transpose(1, 2, 0, 3)
        arr = arr.reshape(nsb * 2 * KB, VW)[:nlq]
        l = arr[:, D]
        rows = arr[:, :D] / np.where(l > 0, l, 1.0)[:, None]
        nz = np.flatnonzero(vm[b])
        first = nz[0] if nz.size else T
        fix = np.flatnonzero(Lq < first)
        if fix.size:
            rr = Lq[fix]
            s = ((q[b, rr] @ k[b].T) * scale).astype(np.float32)
            s = s - np.float32(NEG_BIG)
            s = s.astype(np.float64)
            s -= s.max(axis=1, keepdims=True)
            p = np.exp(s)
            p /= p.sum(axis=1, keepdims=True)
            rows[fix] = (p @ v[b].astype(np.float64)).astype(np.float32)
        out[b][Lq] = rows
    return out


def kernel(**inputs):
    from concourse.bass_utils import run_bass_kernel_spmd

    in_maps, key = _host_inputs(**inputs)
    nc = _get_nc(key)
    res = run_bass_kernel_spmd(nc, in_maps, list(range(NCORES))).results
    return _host_gather(res, **inputs)


# revision 5
# speedup vs baseline: 1.0425x; 1.0425x over previous
"""v4: query-compacted + key-compacted sparse causal attention.

Sharding: 8 cores = 4 batches x 2 key-parity shards.  Host compacts BOTH
the live queries (q_mask kills ~50% of output rows -> never computed) and
the live keys (v_mask, split even/odd between the 2 cores of a batch).

Device per core:
  scores  s[kb] = kt[:,kb]T @ qt[:,sb]    (f32r, out [128k, 256q])
  exp     p = exp(s)                       (Act, bf16 out)
  causal  p *= (col_iota >= jthr)          (stt on DVE; jthr precomputed on
                                            host in column-index space)
  PV      o[128q, 65] += p_subT @ vp[kb]   (bf16 moving, out free = 65)
Output per superblock: [128q, 2*65] = (v-dot | l) for 2 query subblocks;
host sums the 2 parity cores, divides by l, scatters to live rows.

Pipeline: GRP=4 key blocks per exp call, 3 score PSUM buffers, PV deferred
two groups so the Act engine streams without waiting on masks/PV.
"""

import sys
from collections import deque

import numpy as np

try:
    import concourse  # noqa: F401
except ImportError:  # pragma: no cover
    sys.path.insert(0, "/opt/trn_rl_repo")

import ml_dtypes

B, T, D = 4, 4096, 64
NCORES = 8
QSB = 256
KB = 128
VW = 65
GRP = 4
PIPE = 2
NEG_BIG = 1e9

_compiled = {}
_last_plan = None


def _build_nc(key):
    import concourse.bass as bass
    import concourse.mybir as mybir
    import concourse.tile as tile
    from concourse import bacc

    nsb, blocks, slots = key
    f32 = mybir.dt.float32
    f32r = mybir.dt.float32r
    bf16 = mybir.dt.bfloat16
    nb_tot = blocks[-1]
    Tqpad = nsb * QSB
    ns_tot = sum(len(s) for s in slots)
    thw = QSB + max(1, ns_tot)          # col-iota | per-slot thresholds

    nc = bacc.Bacc(None, target_bir_lowering=False, debug=False)
    qt_d = nc.declare_dram_parameter("qt", [D, Tqpad], f32r, isOutput=False)
    kt_d = nc.declare_dram_parameter("kt", [D, nb_tot * KB], f32r,
                                     isOutput=False)
    vp_d = nc.declare_dram_parameter("vp", [KB, nb_tot * VW], bf16,
                                     isOutput=False)
    th_d = nc.declare_dram_parameter("th", [KB, thw], f32, isOutput=False)
    o_d = nc.declare_dram_parameter("o", [KB, nsb * 2 * VW], f32,
                                    isOutput=True)

    thcol = {}
    c = 0
    for i in range(nsb):
        for (kb, _chi) in slots[i]:
            thcol[(i, kb)] = QSB + c
            c += 1

    with tile.TileContext(nc) as tc:
        with (
            tc.tile_pool(name="const", bufs=1) as cpool,
            tc.tile_pool(name="pt", bufs=6) as ppool,
            tc.tile_pool(name="ob", bufs=3) as obpool,
            tc.tile_pool(name="ps", bufs=3, space=bass.MemorySpace.PSUM) as spool,
            tc.tile_pool(name="po", bufs=2, space=bass.MemorySpace.PSUM) as opool,
        ):
            qt = cpool.tile([D, Tqpad], f32r)
            kt = cpool.tile([D, nb_tot * KB], f32r)
            vp = cpool.tile([KB, nb_tot * VW], bf16)
            th = cpool.tile([KB, thw], f32)

            # order DMAs so early superblocks can start quickly
            nA = min(2, nsb)
            ktA = blocks[nA - 1] * KB
            nc.sync.dma_start(kt[:, 0:ktA], kt_d[:, 0:ktA])
            nc.sync.dma_start(qt[:, 0:nA * QSB], qt_d[:, 0:nA * QSB])
            nc.sync.dma_start(th[:], th_d[:])
            nc.sync.dma_start(vp[:], vp_d[:])
            if ktA < nb_tot * KB:
                nc.sync.dma_start(kt[:, ktA:], kt_d[:, ktA:])
            if nsb > nA:
                nc.sync.dma_start(qt[:, nA * QSB:], qt_d[:, nA * QSB:])

            # (superblock, first kb, group size, total blocks)
            items = []
            for i in range(nsb):
                nb = blocks[i]
                ng = -(-nb // GRP)
                bsz, rem = divmod(nb, ng)
                sizes = [bsz + 1] * rem + [bsz] * (ng - rem)
                kb0 = 0
                for sz in sizes:
                    items.append((i, kb0, sz, nb))
                    kb0 += sz

            o_ps = {}
            pending = deque()

            def emit_pv(i, kb0, sz, nb, p):
                for j in range(sz):
                    kb = kb0 + j
                    for sub in range(2):
                        nc.tensor.matmul(
                            o_ps[i][:, sub * VW:(sub + 1) * VW],
                            p[:, j * QSB + sub * KB:j * QSB + (sub + 1) * KB],
                            vp[:, kb * VW:(kb + 1) * VW],
                            # start zeroes the whole 2KB zero-region (bank):
                            # one accumulation group per o tile
                            start=(kb == 0 and sub == 0),
                            stop=(kb == nb - 1 and sub == 1),
                        )
                if kb0 + sz == nb:
                    ob = obpool.tile([KB, 2 * VW], f32, name=f"ob{i}",
                                     tag="ob")
                    nc.vector.tensor_copy(ob[:], o_ps[i][:])
                    nc.sync.dma_start(o_d[:, i * 2 * VW:(i + 1) * 2 * VW],
                                      ob[:])
                    del o_ps[i]

            for (i, kb0, sz, nb) in items:
                if kb0 == 0:
                    o_ps[i] = opool.tile([KB, 2 * VW], f32,
                                         name=f"oacc{i}", tag="oacc")
                w = sz * QSB
                s = spool.tile([KB, GRP * QSB], f32)
                for j in range(sz):
                    kb = kb0 + j
                    nc.tensor.matmul(
                        s[:, j * QSB:(j + 1) * QSB],
                        kt[:, kb * KB:(kb + 1) * KB],
                        qt[:, i * QSB:(i + 1) * QSB],
                        start=True, stop=True,
                    )
                if len(pending) >= PIPE:
                    emit_pv(*pending.popleft())
                p = ppool.tile([KB, GRP * QSB], bf16)
                nc.scalar.activation(
                    p[:, 0:w], s[:, 0:w],
                    mybir.ActivationFunctionType.Exp,
                )
                for (kb, chi) in slots[i]:
                    if kb0 <= kb < kb0 + sz:
                        j = kb - kb0
                        col = thcol[(i, kb)]
                        nc.vector.scalar_tensor_tensor(
                            p[:, j * QSB:j * QSB + chi],
                            th[:, 0:chi],
                            th[:, col:col + 1],
                            p[:, j * QSB:j * QSB + chi],
                            op0=mybir.AluOpType.is_ge,
                            op1=mybir.AluOpType.mult,
                        )
                pending.append((i, kb0, sz, nb, p))
            while pending:
                emit_pv(*pending.popleft())

    nc.compile()
    return nc


def _get_nc(key):
    if key not in _compiled:
        _compiled[key] = _build_nc(key)
    return _compiled[key]


def _host_inputs(query, value, keys, q_mask, v_mask, scale):
    global _last_plan
    scale = np.float32(scale)
    q = np.asarray(query, np.float32)
    v = np.asarray(value, np.float32)
    k = np.asarray(keys, np.float32)
    qm = np.asarray(q_mask).astype(bool)
    vm = np.asarray(v_mask).astype(bool)

    Lqs = [np.flatnonzero(qm[b]) for b in range(B)]
    nlqs = [len(x) for x in Lqs]
    Tqpad = max(QSB, -(-max(nlqs) // QSB) * QSB)
    nsb = Tqpad // QSB
    lives = []
    for c in range(NCORES):
        b, par = c // 2, c % 2
        lives.append(np.flatnonzero(vm[b])[par::2])

    # per-core packed keys: only those visible to some live query
    packed = []
    for c in range(NCORES):
        b = c // 2
        live = lives[c]
        ncnt = int(np.searchsorted(live, Lqs[b][-1] + 1)) if nlqs[b] else 0
        packed.append(live[:ncnt])

    # per-core, per-superblock base/top q_orig and key count
    base = np.full((NCORES, nsb), -1, np.int64)      # -1: no real cols
    cnt = np.zeros((NCORES, nsb), np.int64)
    for c in range(NCORES):
        b = c // 2
        Lq, nlq = Lqs[b], nlqs[b]
        for i in range(nsb):
            j0 = i * QSB
            if j0 < nlq:
                base[c, i] = Lq[j0]
                top = Lq[min((i + 1) * QSB, nlq) - 1]
                cnt[c, i] = np.searchsorted(packed[c], top + 1)
            else:
                cnt[c, i] = cnt[c, i - 1] if i else 0

    blocks = []
    for i in range(nsb):
        nb = max(1, int(max(-(-cnt[c, i] // KB) for c in range(NCORES))))
        if blocks:
            nb = max(nb, blocks[-1])
        blocks.append(nb)
    nb_tot = blocks[-1]
    npad = nb_tot * KB

    # causal-boundary slots: walk blocks from the top; a block is clean when
    # for every core all its real keys are <= that core's superblock base.
    slots = []
    for i in range(nsb):
        sl = []
        for kb in range(blocks[i] - 1, -1, -1):
            allvis = True
            chi = 0
            for c in range(NCORES):
                b = c // 2
                seg = packed[c][kb * KB:(kb + 1) * KB]
                if seg.size == 0 or base[c, i] < 0:
                    continue
                kmax = int(seg[-1])
                if kmax > base[c, i]:
                    allvis = False
                Lq_sb = Lqs[b][i * QSB:min((i + 1) * QSB, nlqs[b])]
                chi = max(chi, int(np.searchsorted(Lq_sb, kmax)))
            if chi > 0:
                sl.append((kb, chi))
            if allvis:
                break
        slots.append(tuple(reversed(sl)))
    slots = tuple(slots)
    key = (nsb, tuple(blocks), slots)

    ns_tot = sum(len(s) for s in slots)
    thw = QSB + max(1, ns_tot)
    in_maps = []
    for c in range(NCORES):
        b = c // 2
        pk = packed[c]
        ncnt = len(pk)
        k_orig = np.full(npad, T, np.int64)
        k_orig[:ncnt] = pk
        kc = np.zeros((npad, D), np.float32)
        kc[:ncnt] = k[b][pk]
        vc = np.zeros((npad, VW), np.float32)
        vc[:ncnt, :D] = v[b][pk]
        vc[:ncnt, D] = 1.0
        qt = np.zeros((D, Tqpad), np.float32)
        if nlqs[b]:
            qt[:, :nlqs[b]] = (q[b][Lqs[b]] * scale).T
        kt = np.ascontiguousarray(kc.T)
        vp = np.ascontiguousarray(
            vc.reshape(nb_tot, KB, VW).transpose(1, 0, 2).reshape(KB, -1)
        ).astype(ml_dtypes.bfloat16)
        th = np.zeros((KB, thw), np.float32)
        th[:, :QSB] = np.arange(QSB, dtype=np.float32)[None, :]
        col = QSB
        for i in range(nsb):
            Lq_sb = Lqs[b][i * QSB:min((i + 1) * QSB, nlqs[b])]
            for (kb, _chi) in slots[i]:
                # threshold in column-index space: col kept iff its index
                # >= #cols with q_orig < k_orig  (q_orig >= k_orig)
                th[:, col] = np.searchsorted(
                    Lq_sb, k_orig[kb * KB:(kb + 1) * KB]).astype(np.float32)
                col += 1
        in_maps.append({"qt": qt, "kt": kt, "vp": vp, "th": th})

    _last_plan = {"Lqs": Lqs, "lives": lives, "packed": packed,
                  "blocks": blocks, "slots": slots, "nsb": nsb,
                  "Tqpad": Tqpad, "base": base, "cnt": cnt}
    return in_maps, key


def _host_gather(results, query, value, keys, q_mask, v_mask, scale):
    q = np.asarray(query, np.float32)
    v = np.asarray(value, np.float32)
    k = np.asarray(keys, np.float32)
    vm = np.asarray(v_mask).astype(bool)
    scale = np.float32(scale)
    plan = _last_plan
    nsb = plan["nsb"]

    out = np.zeros((B, T, D), np.float32)
    for b in range(B):
        Lq = plan["Lqs"][b]
        nlq = len(Lq)
        if nlq == 0:
            continue
        osum = results[2 * b]["o"].astype(np.float32) \
            + results[2 * b + 1]["o"].astype(np.float32)
        arr = osum.reshape(KB, nsb, 2, VW).transpose(1, 2, 0, 3)
        arr = arr.reshape(nsb * 2 * KB, VW)[:nlq]
        l = arr[:, D]
        rows = arr[:, :D] / np.where(l > 0, l, 1.0)[:, None]
        nz = np.flatnonzero(vm[b])
        first = nz[0] if nz.size else T
        fix = np.flatnonzero(Lq < first)
        if fix.size:
            rr = Lq[fix]
            s = ((q[b, rr] @ k[b].T) * scale).astype(np.float32)
            s = s - np.float32(NEG_BIG)
            s = s.astype(np.float64)
            s -= s.max(axis=1, keepdims=True)
            p = np.exp(s)
            p /= p.sum(axis=1, keepdims=True)
            rows[fix] = (p @ v[b].astype(np.float64)).astype(np.float32)
        out[b][Lq] = rows
    return out


def kernel(**inputs):
    from concourse.bass_utils import run_bass_kernel_spmd

    in_maps, key = _host_inputs(**inputs)
    nc = _get_nc(key)
    res = run_bass_kernel_spmd(nc, in_maps, list(range(NCORES))).results
    return _host_gather(res, **inputs)


# revision 9
# speedup vs baseline: 1.1805x; 1.1324x over previous
"""v5: query-compacted + key-compacted sparse causal attention.

Sharding: 8 cores = 4 batches x 2 key-parity shards.  Host compacts BOTH
the live queries (q_mask kills ~50% of output rows -> never computed) and
the live keys (v_mask, split even/odd between the 2 cores of a batch).

Device per core:
  scores  s[kb] = kt[:,kb]T @ qt[:,sb]    (f32r, out [128k, W])
  exp     p = exp(s)                       (Act, bf16 out)
  causal  p *= (col_iota >= jthr)          (stt on DVE; jthr precomputed on
                                            host in column-index space)
  PV      o[q, 65] += p_subT @ vp[kb]      (bf16 moving, out free = 65)
Output per superblock: [q, 2*65] = (v-dot | l); host sums the 2 parity
cores, divides by l, scatters to live rows.

Superblocks are 256 query-columns except the last, which is trimmed to a
64-multiple to avoid exp'ing padding.  PE is warmed up with dummy matmuls
during the DMA fill so real scores run at full clock.  PV is deferred two
groups so the Act engine streams without waiting on masks/PV.
"""

import sys
from collections import deque

import numpy as np

try:
    import concourse  # noqa: F401
except ImportError:  # pragma: no cover
    sys.path.insert(0, "/opt/trn_rl_repo")

import ml_dtypes

B, T, D = 4, 4096, 64
NCORES = 8
QSB = 256
KB = 128
VW = 65
GCOLS = 1024          # max score/exp group width (PSUM tile cols)
PIPE = 2
NWARM = 12
NEG_BIG = 1e9

_compiled = {}
_last_plan = None


def _build_nc(key):
    import concourse.bass as bass
    import concourse.mybir as mybir
    import concourse.tile as tile
    from concourse import bacc

    widths, blocks, slots = key
    nsb = len(widths)
    qoff = [0]
    for w_ in widths:
        qoff.append(qoff[-1] + w_)
    Tqpad = qoff[-1]
    f32 = mybir.dt.float32
    f32r = mybir.dt.float32r
    bf16 = mybir.dt.bfloat16
    nb_tot = blocks[-1]
    ns_tot = sum(len(s) for s in slots)
    thw = QSB + max(1, ns_tot)          # col-iota | per-slot thresholds

    nc = bacc.Bacc(None, target_bir_lowering=False, debug=False)
    qt_d = nc.declare_dram_parameter("qt", [D, Tqpad], f32r, isOutput=False)
    kt_d = nc.declare_dram_parameter("kt", [D, nb_tot * KB], f32r,
                                     isOutput=False)
    vp_d = nc.declare_dram_parameter("vp", [KB, nb_tot * VW], bf16,
                                     isOutput=False)
    th_d = nc.declare_dram_parameter("th", [KB, thw], f32, isOutput=False)
    o_d = nc.declare_dram_parameter("o", [KB, nsb * 2 * VW], f32,
                                    isOutput=True)

    thcol = {}
    c = 0
    for i in range(nsb):
        for (kb, _chi) in slots[i]:
            thcol[(i, kb)] = QSB + c
            c += 1

    with tile.TileContext(nc) as tc:
        with (
            tc.tile_pool(name="const", bufs=1) as cpool,
            tc.tile_pool(name="pt", bufs=6) as ppool,
            tc.tile_pool(name="ob", bufs=3) as obpool,
            tc.tile_pool(name="ps", bufs=3, space=bass.MemorySpace.PSUM) as spool,
            tc.tile_pool(name="po", bufs=2, space=bass.MemorySpace.PSUM) as opool,
        ):
            qt = cpool.tile([D, Tqpad], f32r)
            kt = cpool.tile([D, nb_tot * KB], f32r)
            vp = cpool.tile([KB, nb_tot * VW], bf16)
            th = cpool.tile([KB, thw], f32)
            warm = cpool.tile([KB, QSB], bf16)

            # PE warm-up: matmuls on a memset tile while input DMAs stream,
            # so the PE p-state ramps to full clock before real scores.
            nc.vector.memset(warm[:], 0.0)
            for _ in range(NWARM):
                sw = spool.tile([KB, GCOLS], f32, name="swarm", tag="s")
                nc.tensor.matmul(sw[:, 0:QSB], warm[:, 0:KB], warm[:, 0:QSB],
                                 start=True, stop=True)

            # DMA order: first superblock's needs first, then the rest in
            # roughly the order compute consumes them.
            kA = blocks[0] * KB
            qA = qoff[1]
            nc.sync.dma_start(kt[:, 0:kA], kt_d[:, 0:kA])
            nc.sync.dma_start(qt[:, 0:qA], qt_d[:, 0:qA])
            if kA < nb_tot * KB:
                nc.sync.dma_start(kt[:, kA:], kt_d[:, kA:])
            qM = qoff[min(5, nsb)]
            if qM > qA:
                nc.sync.dma_start(qt[:, qA:qM], qt_d[:, qA:qM])
            nc.sync.dma_start(th[:], th_d[:])
            if Tqpad > qM:
                nc.sync.dma_start(qt[:, qM:], qt_d[:, qM:])
            nc.sync.dma_start(vp[:], vp_d[:])

            # (superblock, first kb, group size, total blocks)
            items = []
            for i in range(nsb):
                nb = blocks[i]
                gmax = max(1, GCOLS // widths[i])
                ng = -(-nb // gmax)
                bsz, rem = divmod(nb, ng)
                sizes = [bsz + 1] * rem + [bsz] * (ng - rem)
                kb0 = 0
                for sz in sizes:
                    items.append((i, kb0, sz, nb))
                    kb0 += sz

            o_ps = {}
            pending = deque()

            def emit_pv(i, kb0, sz, nb, p):
                w_ = widths[i]
                nsub = -(-w_ // KB)
                for j in range(sz):
                    kb = kb0 + j
                    for sub in range(nsub):
                        pw = min(KB, w_ - sub * KB)
                        nc.tensor.matmul(
                            o_ps[i][0:pw, sub * VW:(sub + 1) * VW],
                            p[:, j * w_ + sub * KB:j * w_ + sub * KB + pw],
                            vp[:, kb * VW:(kb + 1) * VW],
                            # start zeroes the whole 2KB zero-region (bank):
                            # one accumulation group per o tile
                            start=(kb == 0 and sub == 0),
                            stop=(kb == nb - 1 and sub == nsub - 1),
                        )
                if kb0 + sz == nb:
                    ow = nsub * VW
                    ob = obpool.tile([KB, 2 * VW], f32, name=f"ob{i}",
                                     tag="ob")
                    if w_ >= nsub * KB:
                        nc.vector.tensor_copy(ob[:, 0:ow], o_ps[i][:, 0:ow])
                        nc.sync.dma_start(
                            o_d[:, i * 2 * VW:i * 2 * VW + ow], ob[:, 0:ow])
                    else:
                        # last sub-block covers < 128 query rows: touch only
                        # the initialized partition range
                        for sub in range(nsub):
                            pw = min(KB, w_ - sub * KB)
                            cl = slice(sub * VW, (sub + 1) * VW)
                            dl = slice(i * 2 * VW + sub * VW,
                                       i * 2 * VW + (sub + 1) * VW)
                            nc.vector.tensor_copy(ob[0:pw, cl],
                                                  o_ps[i][0:pw, cl])
                            nc.sync.dma_start(o_d[0:pw, dl], ob[0:pw, cl])
                    del o_ps[i]

            for (i, kb0, sz, nb) in items:
                w_ = widths[i]
                if kb0 == 0:
                    o_ps[i] = opool.tile([KB, 2 * VW], f32,
                                         name=f"oacc{i}", tag="oacc")
                s = spool.tile([KB, GCOLS], f32, tag="s")
                for j in range(sz):
                    kb = kb0 + j
                    nc.tensor.matmul(
                        s[:, j * w_:(j + 1) * w_],
                        kt[:, kb * KB:(kb + 1) * KB],
                        qt[:, qoff[i]:qoff[i] + w_],
                        start=True, stop=True,
                    )
                if len(pending) >= PIPE:
                    emit_pv(*pending.popleft())
                p = ppool.tile([KB, GCOLS], bf16)
                nc.scalar.activation(
                    p[:, 0:sz * w_], s[:, 0:sz * w_],
                    mybir.ActivationFunctionType.Exp,
                )
                for (kb, chi) in slots[i]:
                    if kb0 <= kb < kb0 + sz:
                        j = kb - kb0
                        col = thcol[(i, kb)]
                        nc.vector.scalar_tensor_tensor(
                            p[:, j * w_:j * w_ + chi],
                            th[:, 0:chi],
                            th[:, col:col + 1],
                            p[:, j * w_:j * w_ + chi],
                            op0=mybir.AluOpType.is_ge,
                            op1=mybir.AluOpType.mult,
                        )
                pending.append((i, kb0, sz, nb, p))
            while pending:
                emit_pv(*pending.popleft())

    nc.compile()
    return nc


def _get_nc(key):
    if key not in _compiled:
        _compiled[key] = _build_nc(key)
    return _compiled[key]


def _host_inputs(query, value, keys, q_mask, v_mask, scale):
    global _last_plan
    scale = np.float32(scale)
    q = np.asarray(query, np.float32)
    v = np.asarray(value, np.float32)
    k = np.asarray(keys, np.float32)
    qm = np.asarray(q_mask).astype(bool)
    vm = np.asarray(v_mask).astype(bool)

    Lqs = [np.flatnonzero(qm[b]) for b in range(B)]
    nlqs = [len(x) for x in Lqs]
    maxq = max(nlqs)
    # 256-wide superblocks, last trimmed to a 64-multiple
    nfull = maxq // QSB
    rem = maxq - nfull * QSB
    widths = [QSB] * nfull + ([max(64, -(-rem // 64) * 64)] if rem else [])
    if not widths:
        widths = [64]
    widths = tuple(widths)
    nsb = len(widths)
    qoff = [0]
    for w_ in widths:
        qoff.append(qoff[-1] + w_)
    Tqpad = qoff[-1]

    lives = []
    for c in range(NCORES):
        b, par = c // 2, c % 2
        lives.append(np.flatnonzero(vm[b])[par::2])

    # per-core packed keys: only those visible to some live query
    packed = []
    for c in range(NCORES):
        b = c // 2
        live = lives[c]
        ncnt = int(np.searchsorted(live, Lqs[b][-1] + 1)) if nlqs[b] else 0
        packed.append(live[:ncnt])

    # per-core, per-superblock base/top q_orig and key count
    base = np.full((NCORES, nsb), -1, np.int64)      # -1: no real cols
    cnt = np.zeros((NCORES, nsb), np.int64)
    for c in range(NCORES):
        b = c // 2
        Lq, nlq = Lqs[b], nlqs[b]
        for i in range(nsb):
            j0 = qoff[i]
            if j0 < nlq:
                base[c, i] = Lq[j0]
                top = Lq[min(qoff[i + 1], nlq) - 1]
                cnt[c, i] = np.searchsorted(packed[c], top + 1)
            else:
                cnt[c, i] = cnt[c, i - 1] if i else 0

    blocks = []
    for i in range(nsb):
        nb = max(1, int(max(-(-cnt[c, i] // KB) for c in range(NCORES))))
        if blocks:
            nb = max(nb, blocks[-1])
        blocks.append(nb)
    nb_tot = blocks[-1]
    npad = nb_tot * KB

    # causal-boundary slots: walk blocks from the top; a block is clean when
    # for every core all its real keys are <= that core's superblock base.
    slots = []
    for i in range(nsb):
        sl = []
        for kb in range(blocks[i] - 1, -1, -1):
            allvis = True
            chi = 0
            for c in range(NCORES):
                b = c // 2
                seg = packed[c][kb * KB:(kb + 1) * KB]
                if seg.size == 0 or base[c, i] < 0:
                    continue
                kmax = int(seg[-1])
                if kmax > base[c, i]:
                    allvis = False
                Lq_sb = Lqs[b][qoff[i]:min(qoff[i + 1], nlqs[b])]
                chi = max(chi, int(np.searchsorted(Lq_sb, kmax)))
            if chi > 0:
                sl.append((kb, chi))
            if allvis:
                break
        slots.append(tuple(reversed(sl)))
    slots = tuple(slots)
    key = (widths, tuple(blocks), slots)

    ns_tot = sum(len(s) for s in slots)
    thw = QSB + max(1, ns_tot)
    in_maps = []
    for c in range(NCORES):
        b = c // 2
        pk = packed[c]
        ncnt = len(pk)
        k_orig = np.full(npad, T, np.int64)
        k_orig[:ncnt] = pk
        kc = np.zeros((npad, D), np.float32)
        kc[:ncnt] = k[b][pk]
        vc = np.zeros((npad, VW), np.float32)
        vc[:ncnt, :D] = v[b][pk]
        vc[:ncnt, D] = 1.0
        qt = np.zeros((D, Tqpad), np.float32)
        if nlqs[b]:
            qt[:, :nlqs[b]] = (q[b][Lqs[b]] * scale).T
        kt = np.ascontiguousarray(kc.T)
        vp = np.ascontiguousarray(
            vc.reshape(nb_tot, KB, VW).transpose(1, 0, 2).reshape(KB, -1)
        ).astype(ml_dtypes.bfloat16)
        th = np.zeros((KB, thw), np.float32)
        th[:, :QSB] = np.arange(QSB, dtype=np.float32)[None, :]
        col = QSB
        for i in range(nsb):
            Lq_sb = Lqs[b][qoff[i]:min(qoff[i + 1], nlqs[b])]
            for (kb, _chi) in slots[i]:
                # threshold in column-index space: col kept iff its index
                # >= #cols with q_orig < k_orig  (q_orig >= k_orig)
                th[:, col] = np.searchsorted(
                    Lq_sb, k_orig[kb * KB:(kb + 1) * KB]).astype(np.float32)
                col += 1
        in_maps.append({"qt": qt, "kt": kt, "vp": vp, "th": th})

    _last_plan = {"Lqs": Lqs, "lives": lives, "packed": packed,
                  "blocks": blocks, "slots": slots, "nsb": nsb,
                  "widths": widths, "qoff": qoff,
                  "Tqpad": Tqpad, "base": base, "cnt": cnt}
    return in_maps, key


def _host_gather(results, query, value, keys, q_mask, v_mask, scale):
    q = np.asarray(query, np.float32)
    v = np.asarray(value, np.float32)
    k = np.asarray(keys, np.float32)
    vm = np.asarray(v_mask).astype(bool)
    scale = np.float32(scale)
    plan = _last_plan
    nsb = plan["nsb"]
    widths, qoff = plan["widths"], plan["qoff"]

    out = np.zeros((B, T, D), np.float32)
    for b in range(B):
        Lq = plan["Lqs"][b]
        nlq = len(Lq)
        if nlq == 0:
            continue
        osum = results[2 * b]["o"].astype(np.float32) \
            + results[2 * b + 1]["o"].astype(np.float32)
        arr = np.empty((qoff[-1], VW), np.float32)
        for i in range(nsb):
            for sub in range(-(-widths[i] // KB)):
                pw = min(KB, widths[i] - sub * KB)
                cols = slice(i * 2 * VW + sub * VW,
                             i * 2 * VW + (sub + 1) * VW)
                r0 = qoff[i] + sub * KB
                arr[r0:r0 + pw] = osum[0:pw, cols]
        arr = arr[:nlq]
        l = arr[:, D]
        rows = arr[:, :D] / np.where(l > 0, l, 1.0)[:, None]
        nz = np.flatnonzero(vm[b])
        first = nz[0] if nz.size else T
        fix = np.flatnonzero(Lq < first)
        if fix.size:
            rr = Lq[fix]
            s = ((q[b, rr] @ k[b].T) * scale).astype(np.float32)
            s = s - np.float32(NEG_BIG)
            s = s.astype(np.float64)
            s -= s.max(axis=1, keepdims=True)
            p = np.exp(s)
            p /= p.sum(axis=1, keepdims=True)
            rows[fix] = (p @ v[b].astype(np.float64)).astype(np.float32)
        out[b][Lq] = rows
    return out


def kernel(**inputs):
    from concourse.bass_utils import run_bass_kernel_spmd

    in_maps, key = _host_inputs(**inputs)
    nc = _get_nc(key)
    res = run_bass_kernel_spmd(nc, in_maps, list(range(NCORES))).results
    return _host_gather(res, **inputs)


# revision 11
# speedup vs baseline: 1.2309x; 1.0426x over previous
"""v5: query-compacted + key-compacted sparse causal attention.

Sharding: 8 cores = 4 batches x 2 key-parity shards.  Host compacts BOTH
the live queries (q_mask kills ~50% of output rows -> never computed) and
the live keys (v_mask, split even/odd between the 2 cores of a batch).

Device per core:
  scores  s[kb] = kt[:,kb]T @ qt[:,sb]    (f32r, out [128k, W])
  exp     p = exp(s)                       (Act, bf16 out)
  causal  p *= (col_iota >= jthr)          (stt on DVE; jthr precomputed on
                                            host in column-index space)
  PV      o[q, 65] += p_subT @ vp[kb]      (bf16 moving, out free = 65)
Output per superblock: [q, 2*65] = (v-dot | l); host sums the 2 parity
cores, divides by l, scatters to live rows.

Superblocks are 256 query-columns except the last, which is trimmed to a
64-multiple to avoid exp'ing padding.  PE is warmed up with dummy matmuls
during the DMA fill so real scores run at full clock.  PV is deferred two
groups so the Act engine streams without waiting on masks/PV.
"""

import sys
from collections import deque

import numpy as np

try:
    import concourse  # noqa: F401
except ImportError:  # pragma: no cover
    sys.path.insert(0, "/opt/trn_rl_repo")

import ml_dtypes

B, T, D = 4, 4096, 64
NCORES = 8
QSB = 256
KB = 128
VW = 65
GCOLS = 1024          # max score/exp group width (PSUM tile cols)
PIPE = 2
NWARM = 12
NEG_BIG = 1e9

_compiled = {}
_last_plan = None


def _build_nc(key):
    import concourse.bass as bass
    import concourse.mybir as mybir
    import concourse.tile as tile
    from concourse import bacc

    widths, blocks, slots = key
    nsb = len(widths)
    qoff = [0]
    for w_ in widths:
        qoff.append(qoff[-1] + w_)
    Tqpad = qoff[-1]
    f32 = mybir.dt.float32
    f32r = mybir.dt.float32r
    bf16 = mybir.dt.bfloat16
    nb_tot = blocks[-1]
    ns_tot = sum(len(s) for s in slots)
    thw = QSB + max(1, ns_tot)          # col-iota | per-slot thresholds

    nc = bacc.Bacc(None, target_bir_lowering=False, debug=False)
    qt_d = nc.declare_dram_parameter("qt", [D, Tqpad], f32r, isOutput=False)
    kt_d = nc.declare_dram_parameter("kt", [D, nb_tot * KB], f32r,
                                     isOutput=False)
    vp_d = nc.declare_dram_parameter("vp", [KB, nb_tot * VW], bf16,
                                     isOutput=False)
    th_d = nc.declare_dram_parameter("th", [KB, thw], f32, isOutput=False)
    o_d = nc.declare_dram_parameter("o", [KB, nsb * 2 * VW], f32,
                                    isOutput=True)

    thcol = {}
    c = 0
    for i in range(nsb):
        for (kb, _chi) in slots[i]:
            thcol[(i, kb)] = QSB + c
            c += 1

    with tile.TileContext(nc) as tc:
        with (
            tc.tile_pool(name="const", bufs=1) as cpool,
            tc.tile_pool(name="pt", bufs=6) as ppool,
            tc.tile_pool(name="ob", bufs=3) as obpool,
            tc.tile_pool(name="ps", bufs=3, space=bass.MemorySpace.PSUM) as spool,
            tc.tile_pool(name="po", bufs=2, space=bass.MemorySpace.PSUM) as opool,
        ):
            qt = cpool.tile([D, Tqpad], f32r)
            kt = cpool.tile([D, nb_tot * KB], f32r)
            vp = cpool.tile([KB, nb_tot * VW], bf16)
            th = cpool.tile([KB, thw], f32)
            warm = cpool.tile([KB, QSB], bf16)

            # PE warm-up: matmuls on a memset tile while input DMAs stream,
            # so the PE p-state ramps to full clock before real scores.
            nc.vector.memset(warm[:], 0.0)
            for _ in range(NWARM):
                sw = spool.tile([KB, GCOLS], f32, name="swarm", tag="s")
                nc.tensor.matmul(sw[:, 0:QSB], warm[:, 0:KB], warm[:, 0:QSB],
                                 start=True, stop=True)

            # DMA order: first two superblocks' needs first, then the rest
            # in roughly the order compute consumes them.
            kA = blocks[min(1, nsb - 1)] * KB
            qA = qoff[min(2, nsb)]
            nc.sync.dma_start(kt[:, 0:kA], kt_d[:, 0:kA])
            nc.sync.dma_start(qt[:, 0:qA], qt_d[:, 0:qA])
            if kA < nb_tot * KB:
                nc.sync.dma_start(kt[:, kA:], kt_d[:, kA:])
            qM = qoff[min(5, nsb)]
            if qM > qA:
                nc.sync.dma_start(qt[:, qA:qM], qt_d[:, qA:qM])
            nc.sync.dma_start(vp[:], vp_d[:])
            nc.sync.dma_start(th[:], th_d[:])
            if Tqpad > qM:
                nc.sync.dma_start(qt[:, qM:], qt_d[:, qM:])

            # (superblock, first kb, group size, total blocks)
            items = []
            for i in range(nsb):
                nb = blocks[i]
                gmax = max(1, GCOLS // widths[i])
                ng = -(-nb // gmax)
                bsz, rem = divmod(nb, ng)
                sizes = [bsz + 1] * rem + [bsz] * (ng - rem)
                kb0 = 0
                for sz in sizes:
                    items.append((i, kb0, sz, nb))
                    kb0 += sz

            o_ps = {}
            pending = deque()

            def emit_pv(i, kb0, sz, nb, p):
                w_ = widths[i]
                nsub = -(-w_ // KB)
                for j in range(sz):
                    kb = kb0 + j
                    for sub in range(nsub):
                        pw = min(KB, w_ - sub * KB)
                        nc.tensor.matmul(
                            o_ps[i][0:pw, sub * VW:(sub + 1) * VW],
                            p[:, j * w_ + sub * KB:j * w_ + sub * KB + pw],
                            vp[:, kb * VW:(kb + 1) * VW],
                            # start zeroes the whole 2KB zero-region (bank):
                            # one accumulation group per o tile
                            start=(kb == 0 and sub == 0),
                            stop=(kb == nb - 1 and sub == nsub - 1),
                        )
                if kb0 + sz == nb:
                    ow = nsub * VW
                    ob = obpool.tile([KB, 2 * VW], f32, name=f"ob{i}",
                                     tag="ob")
                    if w_ >= nsub * KB:
                        nc.vector.tensor_copy(ob[:, 0:ow], o_ps[i][:, 0:ow])
                        nc.sync.dma_start(
                            o_d[:, i * 2 * VW:i * 2 * VW + ow], ob[:, 0:ow])
                    else:
                        # last sub-block covers < 128 query rows: touch only
                        # the initialized partition range
                        for sub in range(nsub):
                            pw = min(KB, w_ - sub * KB)
                            cl = slice(sub * VW, (sub + 1) * VW)
                            dl = slice(i * 2 * VW + sub * VW,
                                       i * 2 * VW + (sub + 1) * VW)
                            nc.vector.tensor_copy(ob[0:pw, cl],
                                                  o_ps[i][0:pw, cl])
                            nc.sync.dma_start(o_d[0:pw, dl], ob[0:pw, cl])
                    del o_ps[i]

            for it, (i, kb0, sz, nb) in enumerate(items):
                w_ = widths[i]
                if kb0 == 0:
                    o_ps[i] = opool.tile([KB, 2 * VW], f32,
                                         name=f"oacc{i}", tag="oacc")
                s = spool.tile([KB, GCOLS], f32, tag="s")
                for j in range(sz):
                    kb = kb0 + j
                    nc.tensor.matmul(
                        s[:, j * w_:(j + 1) * w_],
                        kt[:, kb * KB:(kb + 1) * KB],
                        qt[:, qoff[i]:qoff[i] + w_],
                        start=True, stop=True,
                    )
                # drain the PV pipeline early near the end so the final
                # output DMA chains start during the last exp calls
                pipe = PIPE if it < len(items) - PIPE else len(items) - 1 - it
                while len(pending) > pipe:
                    emit_pv(*pending.popleft())
                p = ppool.tile([KB, GCOLS], bf16)
                nc.scalar.activation(
                    p[:, 0:sz * w_], s[:, 0:sz * w_],
                    mybir.ActivationFunctionType.Exp,
                )
                for (kb, chi) in slots[i]:
                    if kb0 <= kb < kb0 + sz:
                        j = kb - kb0
                        col = thcol[(i, kb)]
                        nc.vector.scalar_tensor_tensor(
                            p[:, j * w_:j * w_ + chi],
                            th[:, 0:chi],
                            th[:, col:col + 1],
                            p[:, j * w_:j * w_ + chi],
                            op0=mybir.AluOpType.is_ge,
                            op1=mybir.AluOpType.mult,
                        )
                pending.append((i, kb0, sz, nb, p))
            while pending:
                emit_pv(*pending.popleft())

    nc.compile()
    return nc


def _get_nc(key):
    if key not in _compiled:
        _compiled[key] = _build_nc(key)
    return _compiled[key]


def _host_inputs(query, value, keys, q_mask, v_mask, scale):
    global _last_plan
    scale = np.float32(scale)
    q = np.asarray(query, np.float32)
    v = np.asarray(value, np.float32)
    k = np.asarray(keys, np.float32)
    qm = np.asarray(q_mask).astype(bool)
    vm = np.asarray(v_mask).astype(bool)

    Lqs = [np.flatnonzero(qm[b]) for b in range(B)]
    nlqs = [len(x) for x in Lqs]
    maxq = max(nlqs)
    # 256-wide superblocks, last trimmed to a 64-multiple
    nfull = maxq // QSB
    rem = maxq - nfull * QSB
    widths = [QSB] * nfull + ([max(64, -(-rem // 64) * 64)] if rem else [])
    if not widths:
        widths = [64]
    widths = tuple(widths)
    nsb = len(widths)
    qoff = [0]
    for w_ in widths:
        qoff.append(qoff[-1] + w_)
    Tqpad = qoff[-1]

    lives = []
    for c in range(NCORES):
        b, par = c // 2, c % 2
        lives.append(np.flatnonzero(vm[b])[par::2])

    # per-core packed keys: only those visible to some live query
    packed = []
    for c in range(NCORES):
        b = c // 2
        live = lives[c]
        ncnt = int(np.searchsorted(live, Lqs[b][-1] + 1)) if nlqs[b] else 0
        packed.append(live[:ncnt])

    # per-core, per-superblock base/top q_orig and key count
    base = np.full((NCORES, nsb), -1, np.int64)      # -1: no real cols
    cnt = np.zeros((NCORES, nsb), np.int64)
    for c in range(NCORES):
        b = c // 2
        Lq, nlq = Lqs[b], nlqs[b]
        for i in range(nsb):
            j0 = qoff[i]
            if j0 < nlq:
                base[c, i] = Lq[j0]
                top = Lq[min(qoff[i + 1], nlq) - 1]
                cnt[c, i] = np.searchsorted(packed[c], top + 1)
            else:
                cnt[c, i] = cnt[c, i - 1] if i else 0

    blocks = []
    for i in range(nsb):
        nb = max(1, int(max(-(-cnt[c, i] // KB) for c in range(NCORES))))
        if blocks:
            nb = max(nb, blocks[-1])
        blocks.append(nb)
    nb_tot = blocks[-1]
    npad = nb_tot * KB

    # causal-boundary slots: walk blocks from the top; a block is clean when
    # for every core all its real keys are <= that core's superblock base.
    slots = []
    for i in range(nsb):
        sl = []
        for kb in range(blocks[i] - 1, -1, -1):
            allvis = True
            chi = 0
            for c in range(NCORES):
                b = c // 2
                seg = packed[c][kb * KB:(kb + 1) * KB]
                if seg.size == 0 or base[c, i] < 0:
                    continue
                kmax = int(seg[-1])
                if kmax > base[c, i]:
                    allvis = False
                Lq_sb = Lqs[b][qoff[i]:min(qoff[i + 1], nlqs[b])]
                chi = max(chi, int(np.searchsorted(Lq_sb, kmax)))
            if chi > 0:
                sl.append((kb, chi))
            if allvis:
                break
        slots.append(tuple(reversed(sl)))
    slots = tuple(slots)
    key = (widths, tuple(blocks), slots)

    ns_tot = sum(len(s) for s in slots)
    thw = QSB + max(1, ns_tot)
    in_maps = []
    for c in range(NCORES):
        b = c // 2
        pk = packed[c]
        ncnt = len(pk)
        k_orig = np.full(npad, T, np.int64)
        k_orig[:ncnt] = pk
        kc = np.zeros((npad, D), np.float32)
        kc[:ncnt] = k[b][pk]
        vc = np.zeros((npad, VW), np.float32)
        vc[:ncnt, :D] = v[b][pk]
        vc[:ncnt, D] = 1.0
        qt = np.zeros((D, Tqpad), np.float32)
        if nlqs[b]:
            qt[:, :nlqs[b]] = (q[b][Lqs[b]] * scale).T
        kt = np.ascontiguousarray(kc.T)
        vp = np.ascontiguousarray(
            vc.reshape(nb_tot, KB, VW).transpose(1, 0, 2).reshape(KB, -1)
        ).astype(ml_dtypes.bfloat16)
        th = np.zeros((KB, thw), np.float32)
        th[:, :QSB] = np.arange(QSB, dtype=np.float32)[None, :]
        col = QSB
        for i in range(nsb):
            Lq_sb = Lqs[b][qoff[i]:min(qoff[i + 1], nlqs[b])]
            for (kb, _chi) in slots[i]:
                # threshold in column-index space: col kept iff its index
                # >= #cols with q_orig < k_orig  (q_orig >= k_orig)
                th[:, col] = np.searchsorted(
                    Lq_sb, k_orig[kb * KB:(kb + 1) * KB]).astype(np.float32)
                col += 1
        in_maps.append({"qt": qt, "kt": kt, "vp": vp, "th": th})

    _last_plan = {"Lqs": Lqs, "lives": lives, "packed": packed,
                  "blocks": blocks, "slots": slots, "nsb": nsb,
                  "widths": widths, "qoff": qoff,
                  "Tqpad": Tqpad, "base": base, "cnt": cnt}
    return in_maps, key


def _host_gather(results, query, value, keys, q_mask, v_mask, scale):
    q = np.asarray(query, np.float32)
    v = np.asarray(value, np.float32)
    k = np.asarray(keys, np.float32)
    vm = np.asarray(v_mask).astype(bool)
    scale = np.float32(scale)
    plan = _last_plan
    nsb = plan["nsb"]
    widths, qoff = plan["widths"], plan["qoff"]

    out = np.zeros((B, T, D), np.float32)
    for b in range(B):
        Lq = plan["Lqs"][b]
        nlq = len(Lq)
        if nlq == 0:
            continue
        osum = results[2 * b]["o"].astype(np.float32) \
            + results[2 * b + 1]["o"].astype(np.float32)
        arr = np.empty((qoff[-1], VW), np.float32)
        for i in range(nsb):
            for sub in range(-(-widths[i] // KB)):
                pw = min(KB, widths[i] - sub * KB)
                cols = slice(i * 2 * VW + sub * VW,
                             i * 2 * VW + (sub + 1) * VW)
                r0 = qoff[i] + sub * KB
                arr[r0:r0 + pw] = osum[0:pw, cols]
        arr = arr[:nlq]
        l = arr[:, D]
        rows = arr[:, :D] / np.where(l > 0, l, 1.0)[:, None]
        nz = np.flatnonzero(vm[b])
        first = nz[0] if nz.size else T
        fix = np.flatnonzero(Lq < first)
        if fix.size:
            rr = Lq[fix]
            s = ((q[b, rr] @ k[b].T) * scale).astype(np.float32)
            s = s - np.float32(NEG_BIG)
            s = s.astype(np.float64)
            s -= s.max(axis=1, keepdims=True)
            p = np.exp(s)
            p /= p.sum(axis=1, keepdims=True)
            rows[fix] = (p @ v[b].astype(np.float64)).astype(np.float32)
        out[b][Lq] = rows
    return out


def kernel(**inputs):
    from concourse.bass_utils import run_bass_kernel_spmd

    in_maps, key = _host_inputs(**inputs)
    nc = _get_nc(key)
    res = run_bass_kernel_spmd(nc, in_maps, list(range(NCORES))).results
    return _host_gather(res, **inputs)


# revision 21
# speedup vs baseline: 1.2523x; 1.0174x over previous
"""v5: query-compacted + key-compacted sparse causal attention.

Sharding: 8 cores = 4 batches x 2 key-parity shards.  Host compacts BOTH
the live queries (q_mask kills ~50% of output rows -> never computed) and
the live keys (v_mask, split even/odd between the 2 cores of a batch).

Device per core:
  scores  s[kb] = kt[:,kb]T @ qt[:,sb]    (f32r, out [128k, W])
  exp     p = exp(s)                       (Act, bf16 out)
  causal  p *= (col_iota >= jthr)          (stt on DVE; jthr precomputed on
                                            host in column-index space)
  PV      o[q, 65] += p_subT @ vp[kb]      (bf16 moving, out free = 65)
Output per superblock: [q, 2*65] = (v-dot | l); host sums the 2 parity
cores, divides by l, scatters to live rows.

Superblocks are 256 query-columns except the last, which is trimmed to a
64-multiple to avoid exp'ing padding.  PE is warmed up with dummy matmuls
during the DMA fill so real scores run at full clock.  PV is deferred two
groups so the Act engine streams without waiting on masks/PV.
"""

import sys
from collections import deque

import numpy as np

try:
    import concourse  # noqa: F401
except ImportError:  # pragma: no cover
    sys.path.insert(0, "/opt/trn_rl_repo")

import ml_dtypes

B, T, D = 4, 4096, 64
NCORES = 8
QSB = 256
KB = 128
VW = 65
GCOLS = 1024          # max score/exp group width (PSUM tile cols)
PIPE = 2
NWARM = 12
NEG_BIG = 1e9

_compiled = {}
_last_plan = None


def _build_nc(key):
    import concourse.bass as bass
    import concourse.mybir as mybir
    import concourse.tile as tile
    from concourse import bacc

    widths, blocks, slots = key
    nsb = len(widths)
    qoff = [0]
    for w_ in widths:
        qoff.append(qoff[-1] + w_)
    Tqpad = qoff[-1]
    f32 = mybir.dt.float32
    f32r = mybir.dt.float32r
    bf16 = mybir.dt.bfloat16
    nb_tot = blocks[-1]
    ns_tot = sum(len(s) for s in slots)
    maxW = max(widths)
    thw = maxW + max(1, ns_tot)         # col-iota | per-slot thresholds
    nsubs = [-(-w_ // KB) for w_ in widths]
    ooff = [0]
    for ns_ in nsubs:
        ooff.append(ooff[-1] + ns_ * VW)

    # head chunk: first two superblocks' keys+queries land in one early DMA
    bA = blocks[min(1, nsb - 1)]
    kA = bA * KB
    qA = qoff[min(2, nsb)]

    nc = bacc.Bacc(None, target_bir_lowering=False, debug=False)
    hd_d = nc.declare_dram_parameter("hd", [D, kA + qA], f32r, isOutput=False)
    qt_d = (nc.declare_dram_parameter("qt", [D, Tqpad - qA], f32r,
                                      isOutput=False)
            if Tqpad > qA else None)
    kt_d = (nc.declare_dram_parameter("kt", [D, (nb_tot - bA) * KB], f32r,
                                      isOutput=False)
            if nb_tot > bA else None)
    vp_d = nc.declare_dram_parameter("vp", [KB, nb_tot * VW], bf16,
                                     isOutput=False)
    th_d = nc.declare_dram_parameter("th", [KB, thw], f32, isOutput=False)
    o_d = nc.declare_dram_parameter("o", [KB, ooff[-1]], f32,
                                    isOutput=True)

    thcol = {}
    c = 0
    for i in range(nsb):
        for (kb, _chi) in slots[i]:
            thcol[(i, kb)] = maxW + c
            c += 1

    with tile.TileContext(nc) as tc:
        with (
            tc.tile_pool(name="const", bufs=1) as cpool,
            tc.tile_pool(name="pt", bufs=6) as ppool,
            tc.tile_pool(name="ob", bufs=3) as obpool,
            tc.tile_pool(name="ps", bufs=3, space=bass.MemorySpace.PSUM) as spool,
            tc.tile_pool(name="po", bufs=2, space=bass.MemorySpace.PSUM) as opool,
        ):
            hd = cpool.tile([D, kA + qA], f32r)
            qt = (cpool.tile([D, Tqpad - qA], f32r, name="qt")
                  if qt_d is not None else None)
            kt = (cpool.tile([D, (nb_tot - bA) * KB], f32r, name="kt")
                  if kt_d is not None else None)
            vp = cpool.tile([KB, nb_tot * VW], bf16)
            th = cpool.tile([KB, thw], f32)
            warm = cpool.tile([KB, QSB], bf16)

            def ktsl(kb):
                if kb < bA:
                    return hd[:, kb * KB:(kb + 1) * KB]
                return kt[:, (kb - bA) * KB:(kb - bA + 1) * KB]

            def qtsl(i):
                if qoff[i + 1] <= qA:
                    return hd[:, kA + qoff[i]:kA + qoff[i + 1]]
                return qt[:, qoff[i] - qA:qoff[i + 1] - qA]

            # PE warm-up: matmuls on a memset tile while input DMAs stream,
            # so the PE p-state ramps to full clock before real scores.
            nc.vector.memset(warm[:], 0.0)
            for _ in range(NWARM):
                sw = spool.tile([KB, GCOLS], f32, name="swarm", tag="s")
                nc.tensor.matmul(sw[:, 0:QSB], warm[:, 0:KB], warm[:, 0:QSB],
                                 start=True, stop=True)

            # DMA order: head chunk (first two superblocks) first, then the
            # rest in roughly the order compute consumes them.
            nc.sync.dma_start(hd[:], hd_d[:])
            if kt_d is not None:
                nc.sync.dma_start(kt[:], kt_d[:])
            qM = qoff[min(5, nsb)]
            if qt_d is not None and qM > qA:
                nc.sync.dma_start(qt[:, 0:qM - qA], qt_d[:, 0:qM - qA])
            nc.sync.dma_start(vp[:], vp_d[:])
            nc.sync.dma_start(th[:], th_d[:])
            if qt_d is not None and Tqpad > qM:
                nc.sync.dma_start(qt[:, qM - qA:], qt_d[:, qM - qA:])

            # (superblock, first kb, group size, total blocks)
            items = []
            for i in range(nsb):
                nb = blocks[i]
                gmax = max(1, GCOLS // widths[i])
                ng = -(-nb // gmax)
                bsz, rem = divmod(nb, ng)
                sizes = [bsz + 1] * rem + [bsz] * (ng - rem)
                kb0 = 0
                for sz in sizes:
                    items.append((i, kb0, sz, nb))
                    kb0 += sz

            o_ps = {}
            pending = deque()

            maxsub = max(nsubs)

            def emit_pv(i, kb0, sz, nb, p):
                w_ = widths[i]
                nsub = nsubs[i]
                for j in range(sz):
                    kb = kb0 + j
                    for sub in range(nsub):
                        pw = min(KB, w_ - sub * KB)
                        nc.tensor.matmul(
                            o_ps[i][0:pw, sub * VW:(sub + 1) * VW],
                            p[:, j * w_ + sub * KB:j * w_ + sub * KB + pw],
                            vp[:, kb * VW:(kb + 1) * VW],
                            # start zeroes the whole 2KB zero-region (bank):
                            # one accumulation group per o tile
                            start=(kb == 0 and sub == 0),
                            stop=(kb == nb - 1 and sub == nsub - 1),
                        )
                if kb0 + sz == nb:
                    ob = obpool.tile([KB, maxsub * VW], f32, name=f"ob{i}",
                                     tag="ob")
                    nfull = w_ // KB
                    if nfull:
                        cw = nfull * VW
                        nc.vector.tensor_copy(ob[:, 0:cw], o_ps[i][:, 0:cw])
                        nc.sync.dma_start(
                            o_d[:, ooff[i]:ooff[i] + cw], ob[:, 0:cw])
                    if nfull < nsub:
                        # trailing sub-block covers < 128 query rows: touch
                        # only the initialized partition range
                        pw = w_ - nfull * KB
                        cl = slice(nfull * VW, (nfull + 1) * VW)
                        dl = slice(ooff[i] + nfull * VW,
                                   ooff[i] + (nfull + 1) * VW)
                        nc.vector.tensor_copy(ob[0:pw, cl],
                                              o_ps[i][0:pw, cl])
                        nc.sync.dma_start(o_d[0:pw, dl], ob[0:pw, cl])
                    del o_ps[i]

            for it, (i, kb0, sz, nb) in enumerate(items):
                w_ = widths[i]
                if kb0 == 0:
                    o_ps[i] = opool.tile([KB, maxsub * VW], f32,
                                         name=f"oacc{i}", tag="oacc")
                s = spool.tile([KB, GCOLS], f32, tag="s")
                for j in range(sz):
                    kb = kb0 + j
                    nc.tensor.matmul(
                        s[:, j * w_:(j + 1) * w_],
                        ktsl(kb),
                        qtsl(i),
                        start=True, stop=True,
                    )
                # drain the PV pipeline early near the end so the final
                # output DMA chains start during the last exp calls
                pipe = PIPE if it < len(items) - PIPE else len(items) - 1 - it
                while len(pending) > pipe:
                    emit_pv(*pending.popleft())
                p = ppool.tile([KB, GCOLS], bf16)
                nc.scalar.activation(
                    p[:, 0:sz * w_], s[:, 0:sz * w_],
                    mybir.ActivationFunctionType.Exp,
                )
                for (kb, chi) in slots[i]:
                    if kb0 <= kb < kb0 + sz:
                        j = kb - kb0
                        col = thcol[(i, kb)]
                        nc.vector.scalar_tensor_tensor(
                            p[:, j * w_:j * w_ + chi],
                            th[:, 0:chi],
                            th[:, col:col + 1],
                            p[:, j * w_:j * w_ + chi],
                            op0=mybir.AluOpType.is_ge,
                            op1=mybir.AluOpType.mult,
                        )
                pending.append((i, kb0, sz, nb, p))
            while pending:
                emit_pv(*pending.popleft())

    nc.compile()
    return nc


def _get_nc(key):
    if key not in _compiled:
        _compiled[key] = _build_nc(key)
    return _compiled[key]


def _host_inputs(query, value, keys, q_mask, v_mask, scale):
    global _last_plan
    scale = np.float32(scale)
    q = np.asarray(query, np.float32)
    v = np.asarray(value, np.float32)
    k = np.asarray(keys, np.float32)
    qm = np.asarray(q_mask).astype(bool)
    vm = np.asarray(v_mask).astype(bool)

    Lqs = [np.flatnonzero(qm[b]) for b in range(B)]
    nlqs = [len(x) for x in Lqs]
    maxq = max(max(nlqs), 64)

    lives = []
    for c in range(NCORES):
        b, par = c // 2, c % 2
        lives.append(np.flatnonzero(vm[b])[par::2])

    # per-core packed keys: only those visible to some live query
    packed = []
    for c in range(NCORES):
        b = c // 2
        live = lives[c]
        ncnt = int(np.searchsorted(live, Lqs[b][-1] + 1)) if nlqs[b] else 0
        packed.append(live[:ncnt])

    # DP over superblock boundaries (64-col granularity): minimize exp work
    # = sum(blocks_i * W_i) + per-call overhead, where blocks_i is the
    # max-over-cores key-block count at the superblock's top query.
    P = -(-maxq // 64)
    nbat = [0] * (P + 1)
    for pos in range(1, P + 1):
        mx = 1
        for c in range(NCORES):
            b = c // 2
            nlq = nlqs[b]
            if nlq == 0:
                continue
            top = Lqs[b][min(pos * 64, nlq) - 1]
            cn = int(np.searchsorted(packed[c], top + 1))
            mx = max(mx, -(-cn // KB))
        nbat[pos] = mx
    OVH = 222               # act per-call overhead in column-equivalents
    INF = float("inf")
    dp = [INF] * (P + 1)
    prev = [0] * (P + 1)
    dp[0] = 0.0
    for j1 in range(1, P + 1):
        # score matmuls must fit a 2KB PSUM bank: W in {256, 512}, and the
        # final (partial) superblock may be {64, 128} as well
        deltas = (4, 8) if j1 < P else (1, 2, 4, 8)
        for dlt in deltas:
            j0 = j1 - dlt
            if j0 < 0 or dp[j0] == INF:
                continue
            w_ = dlt * 64
            nb = nbat[j1]
            ng = -(-nb // max(1, GCOLS // w_))
            cost = dp[j0] + nb * w_ + OVH * ng
            if cost < dp[j1]:
                dp[j1] = cost
                prev[j1] = j0
    widths = []
    j = P
    while j > 0:
        widths.append((j - prev[j]) * 64)
        j = prev[j]
    widths = tuple(reversed(widths))
    nsb = len(widths)
    qoff = [0]
    for w_ in widths:
        qoff.append(qoff[-1] + w_)
    Tqpad = qoff[-1]

    # per-core, per-superblock base/top q_orig and key count
    base = np.full((NCORES, nsb), -1, np.int64)      # -1: no real cols
    cnt = np.zeros((NCORES, nsb), np.int64)
    for c in range(NCORES):
        b = c // 2
        Lq, nlq = Lqs[b], nlqs[b]
        for i in range(nsb):
            j0 = qoff[i]
            if j0 < nlq:
                base[c, i] = Lq[j0]
                top = Lq[min(qoff[i + 1], nlq) - 1]
                cnt[c, i] = np.searchsorted(packed[c], top + 1)
            else:
                cnt[c, i] = cnt[c, i - 1] if i else 0

    blocks = []
    for i in range(nsb):
        nb = max(1, int(max(-(-cnt[c, i] // KB) for c in range(NCORES))))
        if blocks:
            nb = max(nb, blocks[-1])
        blocks.append(nb)
    nb_tot = blocks[-1]
    npad = nb_tot * KB

    # causal-boundary slots: walk blocks from the top; a block is clean when
    # for every core all its real keys are <= that core's superblock base.
    slots = []
    for i in range(nsb):
        sl = []
        for kb in range(blocks[i] - 1, -1, -1):
            allvis = True
            chi = 0
            for c in range(NCORES):
                b = c // 2
                seg = packed[c][kb * KB:(kb + 1) * KB]
                if seg.size == 0 or base[c, i] < 0:
                    continue
                kmax = int(seg[-1])
                if kmax > base[c, i]:
                    allvis = False
                Lq_sb = Lqs[b][qoff[i]:min(qoff[i + 1], nlqs[b])]
                chi = max(chi, int(np.searchsorted(Lq_sb, kmax)))
            if chi > 0:
                sl.append((kb, chi))
            if allvis:
                break
        slots.append(tuple(reversed(sl)))
    slots = tuple(slots)
    key = (widths, tuple(blocks), slots)

    ns_tot = sum(len(s) for s in slots)
    maxW = max(widths)
    thw = maxW + max(1, ns_tot)
    nsubs = [-(-w_ // KB) for w_ in widths]
    ooff = [0]
    for ns_ in nsubs:
        ooff.append(ooff[-1] + ns_ * VW)
    bA = blocks[min(1, nsb - 1)]
    kA = bA * KB
    qA = qoff[min(2, nsb)]
    in_maps = []
    for c in range(NCORES):
        b = c // 2
        pk = packed[c]
        ncnt = len(pk)
        k_orig = np.full(npad, T, np.int64)
        k_orig[:ncnt] = pk
        kc = np.zeros((npad, D), np.float32)
        kc[:ncnt] = k[b][pk]
        vc = np.zeros((npad, VW), np.float32)
        vc[:ncnt, :D] = v[b][pk]
        vc[:ncnt, D] = 1.0
        qt = np.zeros((D, Tqpad), np.float32)
        if nlqs[b]:
            qt[:, :nlqs[b]] = (q[b][Lqs[b]] * scale).T
        kt = np.ascontiguousarray(kc.T)
        vp = np.ascontiguousarray(
            vc.reshape(nb_tot, KB, VW).transpose(1, 0, 2).reshape(KB, -1)
        ).astype(ml_dtypes.bfloat16)
        th = np.zeros((KB, thw), np.float32)
        th[:, :maxW] = np.arange(maxW, dtype=np.float32)[None, :]
        col = maxW
        for i in range(nsb):
            Lq_sb = Lqs[b][qoff[i]:min(qoff[i + 1], nlqs[b])]
            for (kb, _chi) in slots[i]:
                # threshold in column-index space: col kept iff its index
                # >= #cols with q_orig < k_orig  (q_orig >= k_orig)
                th[:, col] = np.searchsorted(
                    Lq_sb, k_orig[kb * KB:(kb + 1) * KB]).astype(np.float32)
                col += 1
        im = {"hd": np.ascontiguousarray(
                  np.concatenate([kt[:, :kA], qt[:, :qA]], axis=1)),
              "vp": vp, "th": th}
        if Tqpad > qA:
            im["qt"] = np.ascontiguousarray(qt[:, qA:])
        if nb_tot > bA:
            im["kt"] = np.ascontiguousarray(kt[:, kA:])
        in_maps.append(im)

    _last_plan = {"Lqs": Lqs, "lives": lives, "packed": packed,
                  "blocks": blocks, "slots": slots, "nsb": nsb,
                  "widths": widths, "qoff": qoff, "ooff": ooff,
                  "nsubs": nsubs,
                  "Tqpad": Tqpad, "base": base, "cnt": cnt}
    return in_maps, key


def _host_gather(results, query, value, keys, q_mask, v_mask, scale):
    q = np.asarray(query, np.float32)
    v = np.asarray(value, np.float32)
    k = np.asarray(keys, np.float32)
    vm = np.asarray(v_mask).astype(bool)
    scale = np.float32(scale)
    plan = _last_plan
    nsb = plan["nsb"]
    widths, qoff = plan["widths"], plan["qoff"]

    out = np.zeros((B, T, D), np.float32)
    for b in range(B):
        Lq = plan["Lqs"][b]
        nlq = len(Lq)
        if nlq == 0:
            continue
        osum = results[2 * b]["o"].astype(np.float32) \
            + results[2 * b + 1]["o"].astype(np.float32)
        ooff = plan["ooff"]
        arr = np.empty((qoff[-1], VW), np.float32)
        for i in range(nsb):
            for sub in range(plan["nsubs"][i]):
                pw = min(KB, widths[i] - sub * KB)
                cols = slice(ooff[i] + sub * VW, ooff[i] + (sub + 1) * VW)
                r0 = qoff[i] + sub * KB
                arr[r0:r0 + pw] = osum[0:pw, cols]
        arr = arr[:nlq]
        l = arr[:, D]
        rows = arr[:, :D] / np.where(l > 0, l, 1.0)[:, None]
        nz = np.flatnonzero(vm[b])
        first = nz[0] if nz.size else T
        fix = np.flatnonzero(Lq < first)
        if fix.size:
            rr = Lq[fix]
            s = ((q[b, rr] @ k[b].T) * scale).astype(np.float32)
            s = s - np.float32(NEG_BIG)
            s = s.astype(np.float64)
            s -= s.max(axis=1, keepdims=True)
            p = np.exp(s)
            p /= p.sum(axis=1, keepdims=True)
            rows[fix] = (p @ v[b].astype(np.float64)).astype(np.float32)
        out[b][Lq] = rows
    return out


def kernel(**inputs):
    from concourse.bass_utils import run_bass_kernel_spmd

    in_maps, key = _host_inputs(**inputs)
    nc = _get_nc(key)
    res = run_bass_kernel_spmd(nc, in_maps, list(range(NCORES))).results
    return _host_gather(res, **inputs)
